# revision 1
# baseline (speedup 1.0000x reference)
"""Distance-aware masking kernel for Trainium2 (8 NeuronCores).

Computes mask[i,j,:] = W2 @ relu(W1 @ [r_i - c_j, |r_i - c_j|] + b1) + b2
for N=4096 nodes, DIM_OUT=8, sharded by rows across 8 cores.

Strategy (per core, 512 rows):
  - All pairwise terms that are linear in (row-features x col-features) are
    computed on the TensorEngine as small-K matmuls with host-precomputed
    basis operands (split into bf16 triples for fp32-grade accuracy):
      S~[p,j] = a_m^2 * (|r_i|^2 - 2 r_i.c_j + |c_j|^2 + eps)   (squared dist)
      V~[p,j] = alpha_m(i) - g_m(j)                             (linear MLP part)
    with partitions packed p = 4*di + m (32 rows x {3 hidden units + pad}).
  - ACT: D = sqrt(S~) = |a_m| * dist;  DVE: u = D*sign(a_m) + V~ (fused
    scalar_tensor_tensor), h = relu(u) -> float32r.
  - TensorEngine mixes 3 hidden units -> 8 outputs with a block-diagonal
    W2 matmul (f32r), output partitions q = 8*di2 + o.
  - PSUM -> SBUF copy (DVE/ACT), DMA to DRAM scratch laid out [i*8+o, j];
    host transposes to [i, j, o], patches the exact diagonal, concatenates.
"""

import sys

sys.path.insert(0, "/opt/trn_rl_repo")

import numpy as np
import ml_dtypes

N = 4096
N_CORES = 8
ROWS = N // N_CORES          # 512 rows per core
IB = 32                      # i-rows per block (x4 slots = 128 partitions)
NB = ROWS // IB              # 16 blocks
J = 512                      # j-tile (columns per tile)
NJ = N // J                  # 8 j-tiles
EPS = 3e-5                   # dist^2 floor; protects sqrt from f32 cancellation
DIM = 3
DIM_OUT = 8

_BF = ml_dtypes.bfloat16

_CACHE = {}


def _split3(x):
    hi = x.astype(_BF)
    r = x - hi.astype(np.float32)
    mid = r.astype(_BF)
    lo = (r - mid.astype(np.float32)).astype(_BF)
    return hi, mid, lo


def _split2(x):
    hi = x.astype(_BF)
    lo = (x - hi.astype(np.float32)).astype(_BF)
    return hi, lo


def _build_program():
    """Build + compile the SPMD Bass program once (shapes are static)."""
    import concourse.bass as bass  # noqa: F401
    import concourse.mybir as mybir
    import concourse.tile as tile
    from concourse import bacc

    nc = bacc.Bacc("TRN2", target_bir_lowering=False, num_devices=N_CORES)

    f32 = mybir.dt.float32
    f16 = mybir.dt.float16
    bf16 = mybir.dt.bfloat16

    sv_lhsT = nc.dram_tensor("sv_lhsT", [46, NB * 128], bf16, kind="ExternalInput").ap()
    sv_rhs = nc.dram_tensor("sv_rhs", [46, 2 * N], bf16, kind="ExternalInput").ap()
    mixw = nc.dram_tensor("mixw", [128, 128], f16, kind="ExternalInput").ap()
    sgn = nc.dram_tensor("sgn", [128, 1], f32, kind="ExternalInput").ap()
    scratch = nc.dram_tensor("scratch", [N, N], f32, kind="ExternalOutput").ap()

    with tile.TileContext(nc) as tc:
        with tc.tile_pool(name="const", bufs=1) as cp, \
             tc.tile_pool(name="work", bufs=4) as wp, \
             tc.tile_pool(name="outp", bufs=6) as op, \
             tc.tile_pool(name="psv", bufs=2, space="PSUM") as psv, \
             tc.tile_pool(name="psm", bufs=2, space="PSUM") as psm:

            t_sv_lhsT = cp.tile([46, NB * 128], bf16, tag="t_sv_lhsT")
            nc.sync.dma_start(t_sv_lhsT[:], sv_lhsT)
            t_sv_rhs = cp.tile([46, 2 * N], bf16, tag="t_sv_rhs")
            nc.sync.dma_start(t_sv_rhs[:], sv_rhs)
            t_mixw = cp.tile([128, 128], f16, tag="t_mixw")
            nc.sync.dma_start(t_mixw[:], mixw)
            t_sgn = cp.tile([128, 1], f32, tag="t_sgn")
            nc.sync.dma_start(t_sgn[:], sgn)

            for b in range(NB):
                lcol = slice(b * 128, b * 128 + 128)
                for jt in range(NJ):
                    jcol = slice(jt * J, (jt + 1) * J)

                    ps_sv = psv.tile([128, 2 * J], f32, tag="ps_sv")
                    for half in range(2):
                        nc.tensor.matmul(
                            ps_sv[:, half * J:(half + 1) * J],
                            t_sv_lhsT[:, lcol],
                            t_sv_rhs[:, (2 * jt + half) * J:(2 * jt + half + 1) * J],
                            start=True, stop=True,
                        )

                    t_d = wp.tile([128, J], f32, tag="t_d")
                    nc.scalar.activation(
                        t_d[:], ps_sv[:, 0:J], mybir.ActivationFunctionType.Sqrt
                    )
                    t_u = wp.tile([128, J], f32, tag="t_u")
                    nc.vector.scalar_tensor_tensor(
                        t_u[:], t_d[:], t_sgn[:], ps_sv[:, J:2 * J],
                        mybir.AluOpType.mult, mybir.AluOpType.add,
                    )
                    t_h = wp.tile([128, J], f16, tag="t_h")
                    nc.vector.tensor_scalar_max(t_h[:], t_u[:], 0.0)

                    t_o = op.tile([128, 2 * J], f32, tag="t_o")
                    ps_o = psm.tile([128, 2 * J], f32, tag="ps_o")
                    for w in range(2):
                        pr = slice(64 * w, 64 * w + 64)
                        nc.tensor.matmul(
                            ps_o[:, w * J:(w + 1) * J], t_mixw[pr, :], t_h[pr, :],
                            start=True, stop=True,
                        )
                    if (b * NJ + jt) % 4 < 3:
                        nc.scalar.copy(t_o[:], ps_o[:])
                    else:
                        nc.vector.tensor_copy(t_o[:], ps_o[:])
                    # scratch rows r = 256*b + 128*w + q  <->  sbuf [q, w*J+j]
                    row0 = b * IB * DIM_OUT
                    dview = scratch[row0:row0 + 256, jcol].rearrange(
                        "(w q) j -> q w j", w=2
                    )
                    nc.sync.dma_start(dview, t_o[:].rearrange("q (w j) -> q w j", w=2))

    nc.compile()
    return nc


def _host_inputs(node_coords, W1, b1, W2, b2):
    """Build per-core input maps (all small host-side numpy work)."""
    coords = node_coords.astype(np.float32)
    W1 = W1.astype(np.float32)
    b1 = b1.astype(np.float32)
    W2 = W2.astype(np.float32)
    b2 = b2.astype(np.float32)

    a = W1[:, 3]                       # [3] dist coefficients
    a2 = a * a
    Wc = W1[:, :3]                     # [3,3] coord coefficients
    g = coords @ Wc.T                  # [N,3]  g_m(j)
    c2 = (coords * coords).sum(1)      # [N]

    # ---- shared rhs bases ----
    s_base_r = np.zeros((5, N), np.float32)
    s_base_r[0:3] = coords.T
    s_base_r[3] = c2
    s_base_r[4] = 1.0

    v_base_r = np.zeros((4, N), np.float32)
    v_base_r[0] = 1.0
    v_base_r[1:4] = g.T

    Rh, Rm, Rl = _split3(s_base_r)
    vRh, vRl = _split2(v_base_r)

    # ---- mix weights (block-diagonal W2), duplicated for both windows ----
    mixw = np.zeros((128, 128), np.float32)
    for w in range(2):
        for di in range(16):
            for m in range(3):
                mixw[64 * w + 4 * di + m, 8 * di + 0:8 * di + 8] = W2[:, m]
    # rows are p = 4*di+m (K side), cols are q = 8*di+o (M side)

    sgn = np.zeros((128, 1), np.float32)
    for m in range(3):
        sgn[m::4, 0] = np.sign(a[m])

    in_maps = []
    for c in range(N_CORES):
        r = coords[c * ROWS:(c + 1) * ROWS]          # [512,3]
        r2 = (r * r).sum(1)                          # [512]
        alpha = r @ Wc.T + b1                        # [512,3]

        # packed column index for local row i (0..511): b*128 + 4*(i%32) + m
        i_idx = np.arange(ROWS)
        col = (i_idx // IB) * 128 + 4 * (i_idx % IB)  # [512] base col (m=0)

        s_base_l = np.zeros((5, NB * 128), np.float32)
        v_base_l = np.zeros((4, NB * 128), np.float32)
        for m in range(3):
            cm = col + m
            s_base_l[0:3, cm] = (-2.0 * a2[m]) * r.T
            s_base_l[3, cm] = a2[m]
            s_base_l[4, cm] = a2[m] * (r2 + EPS)
            v_base_l[0, cm] = alpha[:, m]
            v_base_l[m + 1, cm] = -1.0

        Lh, Lm, Ll = _split3(s_base_l)
        vLh, vLl = _split2(v_base_l)

        # pair order: big (hh) terms first so cancellation happens early
        s_lhsT = np.vstack([Lh, Lh, Lm, Lh, Ll, Lm])      # [30, 2048]
        s_rhs = np.vstack([Rh, Rm, Rh, Rl, Rh, Rm])       # [30, 4096]
        v_lhsT = np.vstack([vLh, vLh, vLl, vLl])          # [16, 2048]
        v_rhs = np.vstack([vRh, vRl, vRh, vRl])           # [16, 4096]

        # merged S|V operands: one K=46 matmul per (block, jtile) computes
        # S in columns [0:J] and V in columns [J:2J] of the psum tile
        sv_lhsT = np.vstack([s_lhsT, v_lhsT])             # [46, 2048]
        sv_rhs = np.zeros((46, 2 * N), _BF)
        sv_view = sv_rhs.reshape(46, NJ, 2, J)
        sv_view[0:30, :, 0, :] = s_rhs.reshape(30, NJ, J)
        sv_view[30:46, :, 1, :] = v_rhs.reshape(16, NJ, J)

        in_maps.append({
            "sv_lhsT": np.ascontiguousarray(sv_lhsT),
            "sv_rhs": np.ascontiguousarray(sv_rhs),
            "mixw": mixw.astype(np.float16),
            "sgn": sgn,
        })
    return in_maps


def kernel(node_coords, W1, b1, W2, b2):
    from concourse.bass_utils import run_bass_kernel_spmd

    if "nc" not in _CACHE:
        _CACHE["nc"] = _build_program()
    nc = _CACHE["nc"]

    in_maps = _host_inputs(node_coords, W1, b1, W2, b2)
    res = run_bass_kernel_spmd(nc, in_maps, core_ids=list(range(N_CORES)))
    _CACHE["last_res"] = res

    out = np.empty((N, N, DIM_OUT), np.float32)
    for c in range(N_CORES):
        sc = res.results[c]["scratch"]                   # [4096, 4096] f16
        blk = sc.reshape(ROWS, DIM_OUT, N).transpose(0, 2, 1)
        out[c * ROWS:(c + 1) * ROWS] = blk

    # b2 is handled here (the device mix omits it)
    if np.any(b2):
        out += b2.astype(np.float32)

    # exact diagonal (pairwise features are exactly zero there; the device
    # path has an eps floor under the sqrt, so patch on host)
    h_diag = np.maximum(b1.astype(np.float32), 0.0)
    diag = W2.astype(np.float32) @ h_diag + b2.astype(np.float32)
    idx = np.arange(N)
    out[idx, idx, :] = diag

    return out



# revision 3
# speedup vs baseline: 1.0147x; 1.0147x over previous
"""Distance-aware masking kernel for Trainium2 (8 NeuronCores).

mask[i,j,:] = W2 @ relu(W1 @ [r_i - c_j, |r_i - c_j|] + b1) + b2,
N=4096 nodes, rows sharded across 8 cores (512 rows each).

Pipeline (per core, 128 group-iterations of 32 rows x 512 cols):
  1. S-matmuls (PE, bf16 split2, K=15): dense dist^2(i,j)+eps, two 32-row
     groups at partition offsets 0/64 of paired [100,1024] psum tiles
     (rows 32..35 / 96..99 reserved for basis rows).
  2. ACT sqrt (batched over the pair): psum -> sbuf bf16 dist; reserved
     rows get sqrt(0)=0, then a small DMA overwrites them with the basis
     rows [1, g0, g1, g2] (g = coords @ Wc.T).
  3. u-matmul (PE, K=36): u[4*di+m, j] = a_m*dist + alpha_m(i) - g_m(j)
     straight into PSUM (alpha rides the "1" basis row, -g on g rows).
  4. DVE relu: psum -> h f16 [128,512] (tensor_scalar_max).
  5. mix matmuls (PE, f16): block-diagonal W2 -> out[8*di+o, j] in PSUM.
  6. psum -> sbuf f16 copy split ACT[0:704]/DVE[704:1024]; one DMA per
     two groups to f16 DRAM scratch laid out row = 8*i + o. Host casts
     to f32, adds b2, and patches the exact diagonal.
Only ACT and DVE can read PSUM on TRN2, so those drains (sqrt + relu +
out-copy = 1792 cols/iter) are the structural bottleneck; the f16 output
(32 MiB/core) keeps the DMA at ~100us, under the drain-bound ~150us.
Emission is software-pipelined (mix of group k-1 after u-matmul of group
k; S-stages of j-tile jt+1 interleaved into jt's tail groups).
"""

import sys

sys.path.insert(0, "/opt/trn_rl_repo")

import numpy as np
import ml_dtypes

N = 4096
N_CORES = 8
ROWS = N // N_CORES          # 512 rows per core
GR = 32                      # rows per group
NG = ROWS // GR              # 16 groups per core
NT = NG // 2                 # 8 rhs tiles per j-tile (2 groups each)
J = 512                      # j-tile width
NJ = N // J                  # 8 j-tiles
EPS = 3e-4                   # dist^2 floor; protects sqrt from cancellation
DIM = 3
DIM_OUT = 8

_BF = ml_dtypes.bfloat16

_CACHE = {}


def _split2(x):
    hi = x.astype(_BF)
    lo = (x - hi.astype(np.float32)).astype(_BF)
    return hi, lo


def _build_program():
    import concourse.bass as bass  # noqa: F401
    import concourse.mybir as mybir
    import concourse.tile as tile
    from concourse import bacc

    nc = bacc.Bacc("TRN2", target_bir_lowering=False, num_devices=N_CORES)

    f16 = mybir.dt.float16
    bf16 = mybir.dt.bfloat16

    s_lhsT = nc.dram_tensor("s_lhsT", [15, NT * 128], bf16, kind="ExternalInput").ap()
    s_rhs = nc.dram_tensor("s_rhs", [15, N], bf16, kind="ExternalInput").ap()
    # basis rows [1,g0,g1,g2] per j-tile, duplicated twice (for the two
    # 512-col halves of a paired rhs tile): basisD[:, jt*1024 : (jt+1)*1024]
    basisD = nc.dram_tensor("basisD", [4, 2 * N], bf16, kind="ExternalInput").ap()
    u_lhsT = nc.dram_tensor("u_lhsT", [100, NG * 128], bf16, kind="ExternalInput").ap()
    mixw = nc.dram_tensor("mixw", [128, 128], f16, kind="ExternalInput").ap()
    scratch = nc.dram_tensor("scratch", [N, N], f16, kind="ExternalOutput").ap()

    NP = NT // 2  # 4 rhs pairs per j-tile

    with tile.TileContext(nc) as tc:
        with tc.tile_pool(name="const", bufs=1) as cp, \
             tc.tile_pool(name="rhsp", bufs=2 * NP) as rp, \
             tc.tile_pool(name="hp", bufs=4) as hp, \
             tc.tile_pool(name="outp", bufs=3) as op, \
             tc.tile_pool(name="pss", bufs=1, space="PSUM") as pss, \
             tc.tile_pool(name="psu", bufs=2, space="PSUM") as psu, \
             tc.tile_pool(name="psm", bufs=2, space="PSUM") as psm:

            t_s_lhsT = cp.tile([15, NT * 128], bf16, tag="t_s_lhsT")
            nc.sync.dma_start(t_s_lhsT[:], s_lhsT)
            t_s_rhs = cp.tile([15, N], bf16, tag="t_s_rhs")
            nc.sync.dma_start(t_s_rhs[:], s_rhs)
            t_u_lhsT = cp.tile([100, NG * 128], bf16, tag="t_u_lhsT")
            nc.sync.dma_start(t_u_lhsT[:], u_lhsT)
            t_mixw = cp.tile([128, 128], f16, tag="t_mixw")
            nc.sync.dma_start(t_mixw[:], mixw)

            rhs_tiles = {}

            def s_stage(jt, p):
                """Paired dist tile p (row-tiles 2p, 2p+1) for j-tile jt:
                two S-matmuls into one [100,1024] psum, one batched sqrt,
                one basis DMA per reserved row band."""
                jcol = slice(jt * J, (jt + 1) * J)
                ps_s = pss.tile([100, 2 * J], mybir.dt.float32, tag="ps_s")
                for k in range(2):
                    t = 2 * p + k
                    nc.tensor.matmul(
                        ps_s[:, k * J:(k + 1) * J],
                        t_s_lhsT[:, t * 128:t * 128 + 100],
                        t_s_rhs[:, jcol],
                        start=True, stop=True,
                    )
                rt = rp.tile([100, 2 * J], bf16, tag="rt")
                nc.scalar.activation(
                    rt[:], ps_s[:], mybir.ActivationFunctionType.Sqrt
                )
                bcol = slice(jt * 2 * J, (jt + 1) * 2 * J)
                nc.gpsimd.dma_start(rt[32:36, :], basisD[:, bcol])
                nc.gpsimd.dma_start(rt[96:100, :], basisD[:, bcol])
                rhs_tiles[(jt, p)] = rt

            def group_front(jt, gi):
                """u-matmul + relu for group gi; returns state for the back half."""
                t, half = gi // 2, gi % 2
                p, kh = t // 2, t % 2
                off = 64 * half
                rt = rhs_tiles[(jt, p)]

                ps_u = psu.tile([128, J], mybir.dt.float32, tag="ps_u")
                nc.tensor.matmul(
                    ps_u[:],
                    t_u_lhsT[off:off + 36, gi * 128:(gi + 1) * 128],
                    rt[off:off + 36, kh * J:(kh + 1) * J],
                    start=True, stop=True,
                )
                t_h = hp.tile([128, J], f16, tag="t_h")
                nc.vector.tensor_scalar_max(t_h[:], ps_u[:], 0.0)
                return (jt, gi, t_h)

            t_o_cur = [None]

            def group_back(state, nd):
                """mix + psum->sbuf copy; out DMA batched per 2 groups."""
                jt, gi, t_h = state
                ps_o = psm.tile([128, 2 * J], mybir.dt.float32, tag="ps_o")
                for w in range(2):
                    pr = slice(64 * w, 64 * w + 64)
                    nc.tensor.matmul(
                        ps_o[:, w * J:(w + 1) * J], t_mixw[pr, :], t_h[pr, :],
                        start=True, stop=True,
                    )
                g = gi % 2
                if g == 0:
                    t_o_cur[0] = op.tile([128, 4 * J], f16, tag="t_o", name="t_o")
                t_o = t_o_cur[0]
                c0 = g * 2 * J
                nc.scalar.copy(t_o[:, c0:c0 + 704], ps_o[:, 0:704])
                nc.vector.tensor_copy(t_o[:, c0 + 704:c0 + 1024], ps_o[:, 704:1024])
                if g == 1:
                    # scratch row = 8*i + o = 256*gi + 128*w + q, two groups
                    jcol = slice(jt * J, (jt + 1) * J)
                    row0 = (gi - 1) * 256
                    dview = scratch[row0:row0 + 512, jcol].rearrange(
                        "(g w q) j -> q g w j", g=2, w=2
                    )
                    src = t_o[:].rearrange("q (g w j) -> q g w j", g=2, w=2)
                    nc.sync.dma_start(dview, src)

            # software-pipelined emission: the mix/copy/DMA of group gi-1 is
            # emitted after the u-matmul+relu of group gi so PE never stalls
            # waiting for the relu; S-stages of jt+1 interleave with the
            # second half of jt's groups.
            for p in range(NP):
                s_stage(0, p)
            prev = None
            nd = 0
            for jt in range(NJ):
                for gi in range(NG):
                    if gi >= NG - NP and jt + 1 < NJ:
                        s_stage(jt + 1, gi - (NG - NP))
                    cur = group_front(jt, gi)
                    if prev is not None:
                        group_back(prev, nd)
                        nd += prev[1] % 2
                    prev = cur
            group_back(prev, nd)

    nc.compile()
    return nc


def _host_inputs(node_coords, W1, b1, W2, b2):
    coords = node_coords.astype(np.float32)
    W1 = W1.astype(np.float32)
    b1 = b1.astype(np.float32)
    W2 = W2.astype(np.float32)

    a = W1[:, 3]                       # [3] dist coefficients
    Wc = W1[:, :3]                     # [3,3] coord coefficients
    g = coords @ Wc.T                  # [N,3]
    c2 = (coords * coords).sum(1)      # [N]

    # ---- S rhs: j-side basis rows [cx, cy, cz, |c|^2, 1], split2 ----
    R = np.zeros((5, N), np.float32)
    R[0:3] = coords.T
    R[3] = c2
    R[4] = 1.0
    Rh, Rl = _split2(R)
    s_rhs = np.vstack([Rh, Rl, Rh])                   # [15, N]

    # ---- basis rows for the u-matmul rhs: [1, g0, g1, g2], each j-tile
    # duplicated twice for the paired [100, 1024] rhs tiles ----
    basis = np.zeros((4, N), np.float32)
    basis[0] = 1.0
    basis[1:4] = g.T
    basisD = np.empty((4, 2 * N), np.float32)
    bv = basisD.reshape(4, NJ, 2, J)
    bv[:, :, 0, :] = basis.reshape(4, NJ, J)
    bv[:, :, 1, :] = basis.reshape(4, NJ, J)
    basisD = basisD.astype(_BF)

    # ---- mix weights (block-diagonal W2), two 64-row windows ----
    mixw = np.zeros((128, 128), np.float32)
    for w in range(2):
        for di in range(16):
            for m in range(3):
                mixw[64 * w + 4 * di + m, 8 * di + 0:8 * di + 8] = W2[:, m]
    mixw = mixw.astype(np.float16)

    in_maps = []
    for c in range(N_CORES):
        r = coords[c * ROWS:(c + 1) * ROWS]          # [512,3]
        r2 = (r * r).sum(1)                          # [512]
        alpha = r @ Wc.T + b1                        # [512,3]

        # ---- S lhsT: per tile t, 2 groups of 32 rows at col offsets 0/64
        L = np.zeros((5, NT * 128), np.float32)
        for t in range(NT):
            for half in range(2):
                i0 = t * 64 + half * 32
                cc = t * 128 + half * 64
                rr = r[i0:i0 + 32]                   # [32,3]
                L[0:3, cc:cc + 32] = -2.0 * rr.T
                L[3, cc:cc + 32] = 1.0
                L[4, cc:cc + 32] = r2[i0:i0 + 32] + EPS
        Lh, Ll = _split2(L)
        s_lhsT = np.vstack([Lh, Lh, Ll])             # [15, NT*128]

        # ---- u lhsT: K rows 0..31 dist, 32 ones, 33..35 g; dup at 64..99
        u = np.zeros((100, NG * 128), np.float32)
        for gi in range(NG):
            i0 = gi * GR
            for di in range(GR):
                for m in range(3):
                    p = gi * 128 + 4 * di + m
                    u[di, p] = a[m]
                    u[32, p] = alpha[i0 + di, m]
                    u[33 + m, p] = -1.0
        u[64:100] = u[0:36]
        u_lhsT = u.astype(_BF)

        in_maps.append({
            "s_lhsT": np.ascontiguousarray(s_lhsT),
            "s_rhs": np.ascontiguousarray(s_rhs),
            "basisD": np.ascontiguousarray(basisD),
            "u_lhsT": np.ascontiguousarray(u_lhsT),
            "mixw": mixw,
        })
    return in_maps


def kernel(node_coords, W1, b1, W2, b2):
    from concourse.bass_utils import run_bass_kernel_spmd

    if "nc" not in _CACHE:
        _CACHE["nc"] = _build_program()
    nc = _CACHE["nc"]

    in_maps = _host_inputs(node_coords, W1, b1, W2, b2)
    res = run_bass_kernel_spmd(nc, in_maps, core_ids=list(range(N_CORES)))
    _CACHE["last_res"] = res

    out = np.empty((N, N, DIM_OUT), np.float32)
    for c in range(N_CORES):
        sc = res.results[c]["scratch"]                   # [4096, 4096] f16
        blk = sc.astype(np.float32).reshape(ROWS, DIM_OUT, N).transpose(0, 2, 1)
        out[c * ROWS:(c + 1) * ROWS] = blk

    b2f = b2.astype(np.float32)
    if np.any(b2f):
        out += b2f

    # exact diagonal (device path has an eps floor under the sqrt)
    h_diag = np.maximum(b1.astype(np.float32), 0.0)
    diag = W2.astype(np.float32) @ h_diag + b2f
    idx = np.arange(N)
    out[idx, idx, :] = diag

    return out
